# revision 3
# baseline (speedup 1.0000x reference)
"""Multi-head attention (B=4, S=2048, D=1024, H=16) on 8 trn2 NeuronCores.

Sharding: data-parallel over batch (4) x tensor-parallel over head-groups (2).
Core c handles batch b=c//2, head group g=c%2 (8 heads each).

Per-core pipeline (all matmuls f32r on PE):
  phase 0: transpose q/k/v on PE -> project to qhT/khT (head-transposed
           [dh, s] layout) and vh (natural [s, dh] layout)
  path N (per head): natural logits [sq,sk] psum -> one big exp (ACT,
           accum_out gives row sums) -> normalize (DVE) -> DMA attn out
  path T (per pair): transposed logits [sk,sq] psum -> exp -> ctx matmuls
           (accumulate over sk) -> normalize via broadcast recip -> ctxT
  out projection: ctxT @ wo -> partial out (host adds the two head groups)
"""
import numpy as np

import concourse.bass as bass
import concourse.mybir as mybir
import concourse.tile as tile
from concourse import bacc
from concourse.bass_utils import run_bass_kernel_spmd

F32 = mybir.dt.float32
F32R = mybir.dt.float32r
AF = mybir.ActivationFunctionType

B, S, D = 4, 2048, 1024
H, DH = 16, 64
HG = 8           # heads per core
N_CORES = 8


def dram_bcast(ap_1d, nparts):
    """Broadcast a 1-D DRAM AP across nparts partitions (step-0 partition)."""
    return bass.AP(tensor=ap_1d.tensor, offset=ap_1d.offset,
                   ap=[[0, nparts]] + list(ap_1d.ap))


def build_bass():
    nc = bacc.Bacc("TRN2", num_devices=N_CORES)

    qx = nc.dram_tensor("qx", [S, D], F32, kind="ExternalInput")
    kx = nc.dram_tensor("kx", [S, D], F32, kind="ExternalInput")
    vx = nc.dram_tensor("vx", [S, D], F32, kind="ExternalInput")
    wq = nc.dram_tensor("wq", [D, 512], F32, kind="ExternalInput")
    wk = nc.dram_tensor("wk", [D, 512], F32, kind="ExternalInput")
    wv = nc.dram_tensor("wv", [D, 512], F32, kind="ExternalInput")
    wo = nc.dram_tensor("wo", [512, D], F32, kind="ExternalInput")
    bq = nc.dram_tensor("bq", [512], F32, kind="ExternalInput")
    bk = nc.dram_tensor("bk", [512], F32, kind="ExternalInput")
    bv = nc.dram_tensor("bv", [512], F32, kind="ExternalInput")
    bo = nc.dram_tensor("bo", [D], F32, kind="ExternalInput")
    ident_d = nc.dram_tensor("ident_d", [128, 128], F32, kind="ExternalInput")

    attn_o = nc.dram_tensor("attn_o", [HG, S, S], F32, kind="ExternalOutput")
    out_o = nc.dram_tensor("out_o", [S, D], F32, kind="ExternalOutput")

    with tile.TileContext(nc) as tc:
        with tc.tile_pool(name="pers", bufs=1) as pers, \
             tc.tile_pool(name="dscr", bufs=1, space="DRAM") as dscr:
            qhT = pers.tile([128, 4, 4, 512], F32R)   # [p, mchunk, sblk, s]
            khT = pers.tile([128, 4, 4, 512], F32R)
            vh = pers.tile([128, 16, 512], F32R)      # [p(s), stile, dcol]
            wo_sb = pers.tile([128, 4, 1024], F32R)   # [p, dcol chunk, dout]
            ident_sb = pers.tile([128, 128], F32)
            bq_sb = pers.tile([128, 4], F32)
            bk_sb = pers.tile([128, 4], F32)
            bvrow = pers.tile([128, 512], F32)
            borow = pers.tile([128, 1024], F32)
            den_sb = pers.tile([128, 16, 8], F32)     # row sums per (sqtile, head)
            recip_sb = pers.tile([128, 16, 8], F32)
            recipT_d = dscr.tile([8, S], F32)         # DRAM scratch, transposed recips

            nc.sync.dma_start(out=ident_sb, in_=ident_d[:, :])
            nc.sync.dma_start(out=wo_sb,
                              in_=wo.rearrange("(c p) n -> p c n", p=128).bitcast(F32R))
            nc.sync.dma_start(out=bq_sb, in_=bq.rearrange("(m p) -> p m", p=128))
            nc.sync.dma_start(out=bk_sb, in_=bk.rearrange("(m p) -> p m", p=128))
            nc.sync.dma_start(out=bvrow, in_=dram_bcast(bv[:], 128))
            nc.sync.dma_start(out=borow, in_=dram_bcast(bo[:], 128))

            # ---------------- phase 0: transposes + projections ----------------
            with tc.tile_pool(name="stage", bufs=4) as stage, \
                 tc.tile_pool(name="w3", bufs=1) as w3, \
                 tc.tile_pool(name="xT", bufs=2) as xTp, \
                 tc.tile_pool(name="ps0", bufs=2, space="PSUM") as ps0, \
                 tc.tile_pool(name="psP", bufs=4, space="PSUM") as psP:
                for xdram, wdram, which in ((kx, wk, "k"), (qx, wq, "q"),
                                            (vx, wv, "v")):
                    w_sb = w3.tile([128, 8, 512], F32R, tag="w")
                    nc.sync.dma_start(
                        out=w_sb,
                        in_=wdram.rearrange("(c p) n -> p c n", p=128).bitcast(F32R))
                    for blk in range(4):
                        sts = []
                        for t in range(4):
                            st = stage.tile([128, 1024], F32, tag="stage")
                            row0 = (4 * blk + t) * 128
                            nc.sync.dma_start(out=st, in_=xdram[row0:row0 + 128, :])
                            sts.append(st)
                        xT = xTp.tile([128, 8, 512], F32R, tag="xT")
                        for c in range(8):
                            pt = ps0.tile([128, 512], F32, tag="pt")
                            for t in range(4):
                                nc.tensor.transpose(
                                    pt[:, 128 * t:128 * (t + 1)],
                                    sts[t][:, 128 * c:128 * (c + 1)], ident_sb)
                            nc.vector.tensor_copy(xT[:, c, :], pt)
                        if which in ("q", "k"):
                            dst = qhT if which == "q" else khT
                            bias = bq_sb if which == "q" else bk_sb
                            for m in range(4):
                                pp = psP.tile([128, 512], F32, tag="pp")
                                for c in range(8):
                                    nc.tensor.matmul(
                                        pp, w_sb[:, c, 128 * m:128 * (m + 1)],
                                        xT[:, c, :], start=(c == 0), stop=(c == 7))
                                nc.vector.tensor_scalar_add(
                                    dst[:, m, blk, :], pp, bias[:, m:m + 1])
                        else:
                            for t in range(4):
                                pp = psP.tile([128, 512], F32, tag="pp")
                                for c in range(8):
                                    nc.tensor.matmul(
                                        pp, xT[:, c, 128 * t:128 * (t + 1)],
                                        w_sb[:, c, :], start=(c == 0), stop=(c == 7))
                                nc.vector.tensor_add(vh[:, 4 * blk + t, :], pp, bvrow)

            # ---------------- attention ----------------
            with tc.tile_pool(name="ctxTp", bufs=1) as ctxTp:
                ctxT = ctxTp.tile([128, 4, 2048], F32R)  # [p(dcol), pair, sq]
                attn_scope = tc.tile_pool(name="apool", bufs=2)
                apool = attn_scope.__enter__()
                epool_cm = tc.tile_pool(name="epool", bufs=3)
                epool = epool_cm.__enter__()
                rpool_cm = tc.tile_pool(name="rpool", bufs=1)
                rpool = rpool_cm.__enter__()
                psN_cm = tc.tile_pool(name="psN", bufs=1, space="PSUM")
                psN = psN_cm.__enter__()
                psL_cm = tc.tile_pool(name="psL", bufs=1, space="PSUM")
                psL = psL_cm.__enter__()
                psC_cm = tc.tile_pool(name="psC", bufs=1, space="PSUM")
                psC = psC_cm.__enter__()
                for p in range(4):
                    h0 = 2 * p
                    # ---- path N: natural logits -> exp -> normalize -> DMA ----
                    for j in range(2):
                        h = h0 + j
                        rh = 64 * j
                        for t in range(16):
                            pn = psN.tile([128, 2048], F32, tag="pn")
                            lhs = qhT[rh:rh + 64, p, t // 4,
                                      128 * (t % 4):128 * (t % 4 + 1)]
                            for sb_ in range(4):
                                nc.tensor.matmul(
                                    pn[:, 512 * sb_:512 * (sb_ + 1)], lhs,
                                    khT[rh:rh + 64, p, sb_, :],
                                    start=True, stop=True)
                            at = apool.tile([128, 2048], F32, tag="attn")
                            nc.scalar.activation(at, pn, AF.Exp, scale=0.125,
                                                 accum_out=den_sb[:, t, h:h + 1])
                            nc.vector.reciprocal(recip_sb[:, t, h:h + 1],
                                                 den_sb[:, t, h:h + 1])
                            nc.vector.tensor_scalar_mul(at, at,
                                                        recip_sb[:, t, h:h + 1])
                            nc.sync.dma_start(
                                out=attn_o[h, 128 * t:128 * (t + 1), :], in_=at)
                    # ---- transposed recips to DRAM, broadcast back ----
                    for j in range(2):
                        nc.sync.dma_start(
                            out=recipT_d.rearrange("h (t p) -> p t h", p=128)[:, :, 2 * p + j],
                            in_=recip_sb[:, :, 2 * p + j])
                    rbc = rpool.tile([128, 2048], F32, tag="rbc")
                    for j in range(2):
                        nc.sync.dma_start(out=rbc[64 * j:64 * (j + 1), :],
                                          in_=dram_bcast(recipT_d[2 * p + j, :], 64))
                    # ---- path T: transposed logits -> exp -> ctx ----
                    for sqb in range(4):
                        pc0 = psC.tile([64, 512], F32, tag="pc0")
                        pc1 = psC.tile([64, 512], F32, tag="pc1")
                        pcs = (pc0, pc1)
                        for skt in range(16):
                            pl = psL.tile([128, 1024], F32, tag="pl")
                            for j in range(2):
                                rh = 64 * j
                                nc.tensor.matmul(
                                    pl[:, 512 * j:512 * (j + 1)],
                                    khT[rh:rh + 64, p, skt // 4,
                                        128 * (skt % 4):128 * (skt % 4 + 1)],
                                    qhT[rh:rh + 64, p, sqb, :],
                                    start=True, stop=True)
                            et = epool.tile([128, 1024], F32R, tag="expT")
                            nc.scalar.activation(et, pl, AF.Exp, scale=0.125)
                            for j in range(2):
                                h = h0 + j
                                nc.tensor.matmul(
                                    pcs[j], vh[:, skt, 64 * h:64 * (h + 1)],
                                    et[:, 512 * j:512 * (j + 1)],
                                    start=(skt == 0), stop=(skt == 15))
                        for j in range(2):
                            nc.vector.tensor_mul(
                                ctxT[64 * j:64 * (j + 1), p, 512 * sqb:512 * (sqb + 1)],
                                pcs[j][:, :], rbc[64 * j:64 * (j + 1),
                                                  512 * sqb:512 * (sqb + 1)])

                for cm in (psC_cm, psL_cm, psN_cm, rpool_cm, epool_cm, attn_scope):
                    cm.__exit__(None, None, None)

                # ---------------- out projection ----------------
                with tc.tile_pool(name="opool", bufs=2) as opool, \
                     tc.tile_pool(name="psO", bufs=2, space="PSUM") as psO:
                    for t in range(16):
                        po = psO.tile([128, 1024], F32, tag="po")
                        for dhalf in range(2):
                            for pch in range(4):
                                nc.tensor.matmul(
                                    po[:, 512 * dhalf:512 * (dhalf + 1)],
                                    ctxT[:, pch, 128 * t:128 * (t + 1)],
                                    wo_sb[:, pch, 512 * dhalf:512 * (dhalf + 1)],
                                    start=(pch == 0), stop=(pch == 3))
                        ob = opool.tile([128, 1024], F32, tag="ob")
                        nc.vector.tensor_add(ob, po, borow)
                        nc.sync.dma_start(out=out_o[128 * t:128 * (t + 1), :],
                                          in_=ob)
    nc.compile()
    return nc


_NC_CACHE = {}


def get_nc():
    if "nc" not in _NC_CACHE:
        _NC_CACHE["nc"] = build_bass()
    return _NC_CACHE["nc"]


def make_in_maps(v, k, q, wq, bq, wk, bk, wv, bv, wo, bo):
    ident = np.eye(128, dtype=np.float32)
    zeros_bo = np.zeros_like(bo)
    in_maps = []
    for c in range(N_CORES):
        b, g = c // 2, c % 2
        cols = slice(512 * g, 512 * (g + 1))
        in_maps.append({
            "qx": np.ascontiguousarray(q[b]),
            "kx": np.ascontiguousarray(k[b]),
            "vx": np.ascontiguousarray(v[b]),
            "wq": np.ascontiguousarray(wq[:, cols]),
            "wk": np.ascontiguousarray(wk[:, cols]),
            "wv": np.ascontiguousarray(wv[:, cols]),
            "wo": np.ascontiguousarray(wo[cols, :]),
            "bq": np.ascontiguousarray(bq[cols]),
            "bk": np.ascontiguousarray(bk[cols]),
            "bv": np.ascontiguousarray(bv[cols]),
            "bo": bo if g == 0 else zeros_bo,
            "ident_d": ident,
        })
    return in_maps


def kernel(v, k, q, wq, bq, wk, bk, wv, bv, wo, bo):
    v, k, q = (np.asarray(x, np.float32) for x in (v, k, q))
    wq, bq, wk, bk = (np.asarray(x, np.float32) for x in (wq, bq, wk, bk))
    wv, bv, wo, bo = (np.asarray(x, np.float32) for x in (wv, bv, wo, bo))

    nc = get_nc()
    in_maps = make_in_maps(v, k, q, wq, bq, wk, bk, wv, bv, wo, bo)
    res = run_bass_kernel_spmd(nc, in_maps, core_ids=list(range(N_CORES)))

    out = np.empty((B, S, D), np.float32)
    attn = np.empty((B, H, S, S), np.float32)
    for c in range(N_CORES):
        b, g = c // 2, c % 2
        attn[b, 8 * g:8 * (g + 1)] = res.results[c]["attn_o"]
        if g == 0:
            out[b] = res.results[c]["out_o"]
        else:
            out[b] += res.results[c]["out_o"]
    return out, attn
